# revision 11
# baseline (speedup 1.0000x reference)
"""Trainium2 Bass kernel: MixedScore MultiHeadAttention.

Math (per batch b, head h):
  S[r,c]   = (q[b,h,r,:] . k[b,h,c,:]) / 4
  t_m[r,c] = a_m*S + c_m*Q + b1_m          (Q = cost_mat[b])
  mixed    = sum_m w2_m * relu(t_m)  (+ b2, dropped: softmax shift-invariant)
  out      = softmax_c(mixed) @ v

Folding |w2_m| into (a_m, c_m, b1_m) gives  mixed = sum_m s_m * relu(A_m*S + C_m*Q + B_m)
with s_m = sign(w2_m), so the w2 multiply disappears.

Layout strategy (per core; core = (b, half-of-heads) shard, 8 heads/core):
  - Everything transposed: S^T tiles (c on partitions, r in free dim).
  - qhi SBUF tensor (128, 8, 512): partitions 0:64 = S^T 64-row c-chunk
    (rewritten per head), partitions 64:128 = cost^T rows (DMA'd once).
    S^T emitted 128-c-wide (4 matmuls/head) then split into two 64-row
    copies on rotating engines.
  - mix1: per (ci, jj): 8 K=128 matmuls (one per g-group of 8 c-values),
    g-PAIRS written into one PSUM tile (128p=(c8,m), 2, 512r).
  - relu with per-partition bias B_m in ONE op per pair (128x1024) on a
    rotating engine (Pool/ACT/DVE), output fp8e4m3 -> paired SBUF tile.
  - mix2: fp8 DoubleRow matmul per pair: lhsT = (128,2,64) sign pattern,
    rhs = (128,2,512) relu pair; 2 k-tiles per pass at 0.5 cyc/row ->
    4x fewer PE cycles than the fp32 version. Accumulates (64c, 512r)
    mixed^T strips in PSUM over the 4 pairs.
  - exp on ACT (no max subtraction: |logit| <= ~21, fp32-safe).
  - PV: lhsT = [v | ones] (c, 17), rhs = exp'd weights (c-chunk, 512r);
    col 16 accumulates the softmax denominator; divide on host.
Matmuls use float32r (full-rate); mix2 uses fp8 DoubleRow (2x rate).
fp8 quantization of relu outputs costs ~8e-3 end-to-end rel err (gate 2e-2).
"""

import itertools
import os
import sys

import ml_dtypes
import numpy as np

sys.path.insert(0, "/opt/trn_rl_repo")

import concourse.bass as bass  # noqa: E402
import concourse.mybir as mybir  # noqa: E402
from concourse import bacc, tile  # noqa: E402
from concourse.bass_utils import run_bass_kernel_spmd  # noqa: E402

FP = mybir.dt.float32
FPR = mybir.dt.float32r
F8 = mybir.dt.float8e4
BF16 = mybir.dt.bfloat16
B, H, R, C, D, M = 4, 16, 512, 512, 16, 16
HPC = 8  # heads per core
NCORES = 8

AF = mybir.ActivationFunctionType
ALU = mybir.AluOpType
DR = mybir.MatmulPerfMode.DoubleRow

last_results = None  # BassKernelResults of the most recent run (for test.py)


def build_bass(mm_dt=FPR, w1_dt=FPR):
    nc = bacc.Bacc(None, target_bir_lowering=False, debug=False)

    qT = nc.declare_dram_parameter("qT", [D, HPC, R], mm_dt, isOutput=False)
    kT = nc.declare_dram_parameter("kT", [D, HPC, C], mm_dt, isOutput=False)
    costT = nc.declare_dram_parameter("costT", [C, R], mm_dt, isOutput=False)
    vx = nc.declare_dram_parameter("vx", [64, HPC, 8, 17], mm_dt, isOutput=False)
    w1s = nc.declare_dram_parameter("w1s", [128, HPC, 8, 128], w1_dt, isOutput=False)
    w2s = nc.declare_dram_parameter("w2s", [128, HPC, 4, 2, 64], F8, isOutput=False)
    bvs = nc.declare_dram_parameter("bvs", [128, HPC], FP, isOutput=False)
    outp = nc.declare_dram_parameter("out", [HPC, D + 1, R], FP, isOutput=True)

    with tile.TileContext(nc) as tc:
        with (
            tc.tile_pool(name="const", bufs=1) as constp,
            tc.tile_pool(name="qhi", bufs=1) as qhip,
            tc.tile_pool(name="r1", bufs=8) as r1p,
            tc.tile_pool(name="wexp", bufs=4) as wexpp,
            tc.tile_pool(name="osb", bufs=4) as osbp,
            tc.tile_pool(name="stg", bufs=3) as stgp,
            tc.tile_pool(name="psS", bufs=1, space="PSUM") as psSp,
            tc.tile_pool(name="ps1", bufs=1, space="PSUM") as ps1p,
            tc.tile_pool(name="psmx", bufs=1, space="PSUM") as psmxp,
            tc.tile_pool(name="pspv", bufs=1, space="PSUM") as pspvp,
        ):
            w1_sb = constp.tile([128, HPC, 8, 128], w1_dt)
            w2_sb = constp.tile([128, HPC, 4, 2, 64], F8)
            bv_sb = constp.tile([128, HPC], FP)
            qT_sb = constp.tile([D, HPC, R], mm_dt)
            kT_sb = constp.tile([D, HPC, C], mm_dt)
            vx_sb = constp.tile([64, HPC, 8, 17], mm_dt)

            qhi = [qhip.tile([128, 8, 512], mm_dt, name=f"qhi{i}", tag=f"qhi{i}") for i in range(2)]
            ps1q = ps1p.tile([128, 4, 512], FP, name="ps1q", tag="ps1q")
            # Preamble DMAs ordered by first use, split across the two HWDGE
            # queues (SP carries cost/qhi, ACT carries weights) so head 0 can
            # start within ~2us instead of waiting on a serial 6 MB preamble.
            nc.sync.dma_start(out=qT_sb[:, 0], in_=qT[:, 0])
            nc.sync.dma_start(out=kT_sb[:, 0], in_=kT[:, 0])
            nc.scalar.dma_start(out=bv_sb[:], in_=bvs[:])
            for g in range(2):
                nc.scalar.dma_start(out=w1_sb[:, 0, g], in_=w1s[:, 0, g])
            for j in range(2):
                nc.sync.dma_start(out=qhi[0][64:128, j, :], in_=costT[64 * j : 64 * j + 64, :])
            for g in range(2, 8):
                nc.scalar.dma_start(out=w1_sb[:, 0, g], in_=w1s[:, 0, g])
            nc.scalar.dma_start(out=w2_sb[:, 0], in_=w2s[:, 0])
            nc.sync.dma_start(out=qT_sb[:, 1], in_=qT[:, 1])
            nc.sync.dma_start(out=kT_sb[:, 1], in_=kT[:, 1])
            for j in range(2, 4):
                nc.sync.dma_start(out=qhi[0][64:128, j, :], in_=costT[64 * j : 64 * j + 64, :])
            nc.scalar.dma_start(out=vx_sb[:, 0], in_=vx[:, 0])
            for j in range(4, 8):
                nc.sync.dma_start(out=qhi[0][64:128, j, :], in_=costT[64 * j : 64 * j + 64, :])
            nc.scalar.dma_start(out=w1_sb[:, 1], in_=w1s[:, 1])
            nc.scalar.dma_start(out=w2_sb[:, 1], in_=w2s[:, 1])
            for j in range(8):
                nc.sync.dma_start(out=qhi[1][64:128, j, :], in_=costT[64 * j : 64 * j + 64, :])
            nc.sync.dma_start(out=qT_sb[:, 2:], in_=qT[:, 2:])
            nc.sync.dma_start(out=kT_sb[:, 2:], in_=kT[:, 2:])
            nc.scalar.dma_start(out=vx_sb[:, 1], in_=vx[:, 1])
            for hh in range(2, HPC):
                nc.scalar.dma_start(out=w1_sb[:, hh], in_=w1s[:, hh])
                nc.scalar.dma_start(out=w2_sb[:, hh], in_=w2s[:, hh])
                nc.scalar.dma_start(out=vx_sb[:, hh], in_=vx[:, hh])

            relu_rr = itertools.cycle(
                [nc.vector, nc.scalar, nc.vector, nc.scalar, nc.vector,
                 nc.scalar, nc.vector, nc.vector, nc.scalar, nc.vector,
                 nc.scalar, nc.vector, nc.scalar, nc.vector, nc.vector,
                 nc.vector]
            )
            stage_rr = itertools.cycle([nc.vector, nc.scalar])
            dma_rr = itertools.cycle([nc.sync, nc.gpsimd])

            def emit_st(hs, jb):
                # one 128-c-wide S^T block for head hs into its qhi buffer
                qdst = qhi[hs % 2]
                ps = psSp.tile([128, 512], FP, name="ps", tag="ps")
                nc.tensor.matmul(
                    ps[:],
                    lhsT=kT_sb[:, hs, 128 * jb : 128 * jb + 128],
                    rhs=qT_sb[:, hs, :],
                    start=True,
                    stop=True,
                )
                stg = stgp.tile([128, 512], mm_dt, name="stg", tag="stg")
                eng = next(stage_rr)
                if eng is nc.scalar:
                    nc.scalar.copy(out=stg[:], in_=ps[:])
                else:
                    eng.tensor_copy(out=stg[:], in_=ps[:])
                for half in range(2):
                    next(dma_rr).dma_start(
                        out=qdst[0:64, 2 * jb + half, :],
                        in_=stg[64 * half : 64 * half + 64, :],
                    )

            def emit_relu(r1slot, p1ap, hh):
                eng = next(relu_rr)
                if eng is nc.scalar:
                    nc.scalar.activation(
                        r1slot, p1ap, AF.Relu, bias=bv_sb[:, hh : hh + 1]
                    )
                else:
                    eng.tensor_scalar(
                        out=r1slot,
                        in0=p1ap,
                        scalar1=bv_sb[:, hh : hh + 1],
                        scalar2=0.0,
                        op0=ALU.add,
                        op1=ALU.max,
                    )

            for jb in range(4):
                emit_st(0, jb)

            # PV matmuls for chunk ci are deferred into chunk ci+1 so the PE
            # never waits on the exp; the head's out-copy rides the last flush.
            hold = {"pend": None, "pvT": None}

            def flush_pv():
                if hold["pend"] is None:
                    return
                hh_, ci_, wx_ = hold["pend"]
                hold["pend"] = None
                if ci_ == 0:
                    hold["pvT"] = pspvp.tile([17, 512], FP, name="pvT", tag="pvT")
                pvT = hold["pvT"]
                for jj in range(2):
                    nc.tensor.matmul(
                        pvT[:],
                        lhsT=vx_sb[:, hh_, 2 * ci_ + jj, :],
                        rhs=wx_[:, jj, :],
                        start=(ci_ == 0 and jj == 0),
                        stop=(ci_ == 3 and jj == 1),
                    )
                if ci_ == 3:
                    ot = osbp.tile([17, 512], FP, name="ot", tag="ot")
                    if hh_ % 2 == 0:
                        nc.vector.tensor_copy(out=ot[:], in_=pvT[:])
                    else:
                        nc.scalar.copy(out=ot[:], in_=pvT[:])
                    nc.sync.dma_start(out=outp[hh_], in_=ot[:])

            for hh in range(HPC):
                qh = qhi[hh % 2]
                for ci in range(4):
                    if hh + 1 < HPC:
                        emit_st(hh + 1, ci)
                    pmx = psmxp.tile([64, 2, 512], FP)
                    r1t = [[None, None, None, None] for _ in range(2)]

                    def emit_mix2(pair):
                        # both jj strips back-to-back: shared w2 stationary
                        for jj in range(2):
                            nc.tensor.matmul(
                                pmx[:, jj, :],
                                lhsT=w2_sb[:, hh, pair, :, :],
                                rhs=r1t[jj][pair][:],
                                start=(pair == 0),
                                stop=(pair == 3),
                                perf_mode=DR,
                            )

                    # mix1 g-pairs fill a persistent 4-bank PSUM ring; one
                    # merged 1024-col relu per (jj, g-pair) halves ACT/DVE
                    # instruction overhead. DoubleRow mix2 lags one pair so
                    # relu latency stays off the PE critical path.
                    for pair in range(4):
                        for jj in range(2):
                            r1t[jj][pair] = r1p.tile(
                                [128, 2, 512], F8, name="r1", tag="r1"
                            )
                        for jj in range(2):
                            for i in range(2):
                                nc.tensor.matmul(
                                    ps1q[:, 2 * jj + i, :],
                                    lhsT=w1_sb[:, hh, 2 * pair + i, :],
                                    rhs=qh[:, 2 * ci + jj, :],
                                    start=True,
                                    stop=True,
                                )
                            emit_relu(
                                r1t[jj][pair][:], ps1q[:, 2 * jj : 2 * jj + 2, :], hh
                            )
                        if pair == 1:
                            flush_pv()
                        if pair >= 1:
                            emit_mix2(pair - 1)
                    emit_mix2(3)
                    wx = wexpp.tile([64, 2, 512], mm_dt, name="wx", tag="wexp")
                    nc.scalar.activation(wx[:], pmx[:], AF.Exp)
                    hold["pend"] = (hh, ci, wx)
            flush_pv()
    _dedupe_weight_loads(nc)
    nc.finalize()
    return nc


def _dedupe_weight_loads(nc):
    """Walk the scheduled PE sequence; when consecutive matmuls use the
    identical stationary AP, mark the later ones ldweights=False so codegen
    skips the redundant LDWEIGHTS (the array still holds those weights)."""
    n = 0
    for bb in nc.m.functions[0].blocks:
        last = None
        for ins in bb.instructions:
            if not isinstance(ins, mybir.InstMatmult):
                continue
            w = ins.ins[1]
            key = (w.memref, w.offset, str(w.ap), str(w.dtype))
            if key == last and ins.ldweights is None:
                ins.ldweights = False
                n += 1
            last = key
    print(f"deduped {n} weight loads", file=sys.stderr)


def prepare_in_maps(q, k, v, cost_mat, mix1_weight, mix1_bias, mix2_weight, mix2_bias):
    q = np.asarray(q, np.float32)
    k = np.asarray(k, np.float32)
    v = np.asarray(v, np.float32)
    cost_mat = np.asarray(cost_mat, np.float32)
    mix1_weight = np.asarray(mix1_weight, np.float32)
    mix1_bias = np.asarray(mix1_bias, np.float32)
    mix2_weight = np.asarray(mix2_weight, np.float32)
    mix2_bias = np.asarray(mix2_bias, np.float32)

    in_maps = []
    for core in range(NCORES):
        b = core // 2
        h0 = (core % 2) * HPC
        qT = np.ascontiguousarray(q[b, h0 : h0 + HPC].transpose(2, 0, 1)) * 0.25
        kT = np.ascontiguousarray(k[b, h0 : h0 + HPC].transpose(2, 0, 1))
        costT = np.ascontiguousarray(cost_mat[b].T)
        vv = v[b, h0 : h0 + HPC]  # (HPC, C, D)
        vxa = np.empty((64, HPC, 8, 17), np.float32)
        vxa[:, :, :, :16] = vv.reshape(HPC, 8, 64, 16).transpose(2, 0, 1, 3)
        vxa[:, :, :, 16] = 1.0

        w1 = mix1_weight[h0 : h0 + HPC]  # (HPC, 2, M)
        b1 = mix1_bias[h0 : h0 + HPC]  # (HPC, M)
        w2 = mix2_weight[h0 : h0 + HPC, :, 0]  # (HPC, M)
        aw = np.abs(w2)
        sg = np.sign(w2).astype(np.float32)
        A = (w1[:, 0, :] * aw).astype(np.float32)  # (HPC, M)
        Cc = (w1[:, 1, :] * aw).astype(np.float32)
        Bb = (b1 * aw).astype(np.float32)

        w1s = np.zeros((128, HPC, 8, 128), np.float32)
        for g in range(8):
            for c8 in range(8):
                cols = slice(c8 * 16, c8 * 16 + 16)
                w1s[8 * g + c8, :, g, cols] = A
                w1s[64 + 8 * g + c8, :, g, cols] = Cc
        w2s = np.zeros((128, HPC, 4, 2, 64), np.float32)
        for pair in range(4):
            for i in range(2):
                g = 2 * pair + i
                for c8 in range(8):
                    w2s[c8 * 16 : c8 * 16 + 16, :, pair, i, 8 * g + c8] = sg.T
        w2s = w2s.astype(ml_dtypes.float8_e4m3)
        bvs = np.tile(Bb.T, (8, 1)).astype(np.float32)  # (128, HPC)

        in_maps.append(
            dict(qT=qT, kT=kT, costT=costT, vx=vxa, w1s=w1s, w2s=w2s, bvs=bvs)
        )
    return in_maps


def assemble(results):
    full = np.empty((B, R, H * D), np.float32)
    for core in range(NCORES):
        b = core // 2
        c0 = (core % 2) * HPC * D
        o = results[core]["out"]  # (HPC, D+1, R); row D is the softmax denom
        o = o[:, :D, :] / o[:, D : D + 1, :]
        full[b, :, c0 : c0 + HPC * D] = o.transpose(2, 0, 1).reshape(R, HPC * D)
    return full


_nc_cache = None


def _install_ntff_hook():
    """The agent image's antenv lacks axon_hooks; recreate it and register
    the ctypes NTFF profiling hook so trace=True yields exec times."""
    import types

    try:
        import antenv

        try:
            import antenv.axon_hooks  # noqa: F401

            return
        except ImportError:
            pass
        mod = types.ModuleType("antenv.axon_hooks")
        mod._hook = None
        mod.set_axon_ntff_profile_hook = lambda h: setattr(mod, "_hook", h)
        mod.get_axon_ntff_profile_hook = lambda: mod._hook
        sys.modules["antenv.axon_hooks"] = mod
        antenv.axon_hooks = mod
        from trn_agent_boot.trn_boot import _ntff_profile_via_ctypes

        mod._hook = _ntff_profile_via_ctypes("/opt/axon/libaxon_pjrt.so")
    except Exception as e:  # profiling is best-effort
        print(f"ntff hook install failed: {e}", file=sys.stderr)


def kernel(**inputs) -> np.ndarray:
    global _nc_cache, last_results
    if _nc_cache is None:
        _nc_cache = build_bass()
    in_maps = prepare_in_maps(**inputs)
    trace = bool(int(os.environ.get("KERNEL_TRACE", "0")))
    if trace:
        _install_ntff_hook()
        import concourse.bass_utils as bu

        bu.upload_artifacts = lambda tmpdir: f"local:{tmpdir}"
    res = run_bass_kernel_spmd(_nc_cache, in_maps, list(range(NCORES)), trace=trace)
    last_results = res
    return assemble(res.results)


# revision 12
# speedup vs baseline: 2.4509x; 2.4509x over previous
"""Trainium2 Bass kernel: MixedScore MultiHeadAttention.

Math (per batch b, head h):
  S[r,c]   = (q[b,h,r,:] . k[b,h,c,:]) / 4
  t_m[r,c] = a_m*S + c_m*Q + b1_m          (Q = cost_mat[b])
  mixed    = sum_m w2_m * relu(t_m)  (+ b2, dropped: softmax shift-invariant)
  out      = softmax_c(mixed) @ v

Folding |w2_m| into (a_m, c_m, b1_m) gives  mixed = sum_m s_m * relu(A_m*S + C_m*Q + B_m)
with s_m = sign(w2_m), so the w2 multiply disappears.

Layout strategy (per core; core = (b, half-of-heads) shard, 8 heads/core):
  - Everything transposed: S^T tiles (c on partitions, r in free dim).
  - qhi SBUF tensor (128, 8, 512): partitions 0:64 = S^T 64-row c-chunk
    (rewritten per head), partitions 64:128 = cost^T rows (DMA'd once).
    S^T emitted 128-c-wide (4 matmuls/head) then split into two 64-row
    copies on rotating engines.
  - mix1: per (ci, jj): 8 K=128 matmuls (one per g-group of 8 c-values),
    g-PAIRS written into one PSUM tile (128p=(c8,m), 2, 512r).
  - relu with per-partition bias B_m in ONE op per pair (128x1024) on a
    rotating engine (Pool/ACT/DVE), output fp8e4m3 -> paired SBUF tile.
  - mix2: fp8 DoubleRow matmul per pair: lhsT = (128,2,64) sign pattern,
    rhs = (128,2,512) relu pair; 2 k-tiles per pass at 0.5 cyc/row ->
    4x fewer PE cycles than the fp32 version. Accumulates (64c, 512r)
    mixed^T strips in PSUM over the 4 pairs.
  - exp on ACT (no max subtraction: |logit| <= ~21, fp32-safe).
  - PV: lhsT = [v | ones] (c, 17), rhs = exp'd weights (c-chunk, 512r);
    col 16 accumulates the softmax denominator; divide on host.
Matmuls use float32r (full-rate); mix2 uses fp8 DoubleRow (2x rate).
fp8 quantization of relu outputs costs ~8e-3 end-to-end rel err (gate 2e-2).
"""

import itertools
import os
import sys

import ml_dtypes
import numpy as np

sys.path.insert(0, "/opt/trn_rl_repo")

import concourse.bass as bass  # noqa: E402
import concourse.mybir as mybir  # noqa: E402
from concourse import bacc, tile  # noqa: E402
from concourse.bass_utils import run_bass_kernel_spmd  # noqa: E402

FP = mybir.dt.float32
FPR = mybir.dt.float32r
F8 = mybir.dt.float8e4
BF16 = mybir.dt.bfloat16
B, H, R, C, D, M = 4, 16, 512, 512, 16, 16
HPC = 8  # heads per core
NCORES = 8

AF = mybir.ActivationFunctionType
ALU = mybir.AluOpType
DR = mybir.MatmulPerfMode.DoubleRow

last_results = None  # BassKernelResults of the most recent run (for test.py)


def build_bass(mm_dt=FPR, w1_dt=FPR):
    nc = bacc.Bacc(None, target_bir_lowering=False, debug=False)

    qT = nc.declare_dram_parameter("qT", [D, HPC, R], mm_dt, isOutput=False)
    kT = nc.declare_dram_parameter("kT", [D, HPC, C], mm_dt, isOutput=False)
    costT = nc.declare_dram_parameter("costT", [C, R], mm_dt, isOutput=False)
    vx = nc.declare_dram_parameter("vx", [64, HPC, 8, 17], mm_dt, isOutput=False)
    w1s = nc.declare_dram_parameter("w1s", [128, HPC, 8, 128], w1_dt, isOutput=False)
    w2s = nc.declare_dram_parameter("w2s", [128, HPC, 4, 2, 64], F8, isOutput=False)
    bvs = nc.declare_dram_parameter("bvs", [128, HPC], FP, isOutput=False)
    outp = nc.declare_dram_parameter("out", [HPC, D + 1, R], FP, isOutput=True)

    with tile.TileContext(nc) as tc:
        with (
            tc.tile_pool(name="const", bufs=1) as constp,
            tc.tile_pool(name="qhi", bufs=1) as qhip,
            tc.tile_pool(name="r1", bufs=8) as r1p,
            tc.tile_pool(name="wexp", bufs=4) as wexpp,
            tc.tile_pool(name="osb", bufs=4) as osbp,
            tc.tile_pool(name="stg", bufs=3) as stgp,
            tc.tile_pool(name="psS", bufs=1, space="PSUM") as psSp,
            tc.tile_pool(name="ps1", bufs=4, space="PSUM") as ps1p,
            tc.tile_pool(name="psmx", bufs=1, space="PSUM") as psmxp,
            tc.tile_pool(name="pspv", bufs=1, space="PSUM") as pspvp,
        ):
            w1_sb = constp.tile([128, HPC, 8, 128], w1_dt)
            w2_sb = constp.tile([128, HPC, 4, 2, 64], F8)
            bv_sb = constp.tile([128, HPC], FP)
            qT_sb = constp.tile([D, HPC, R], mm_dt)
            kT_sb = constp.tile([D, HPC, C], mm_dt)
            vx_sb = constp.tile([64, HPC, 8, 17], mm_dt)

            qhi = [qhip.tile([128, 8, 512], mm_dt, name=f"qhi{i}", tag=f"qhi{i}") for i in range(2)]
            # Preamble DMAs ordered by first use, split across the two HWDGE
            # queues (SP carries cost/qhi, ACT carries weights) so head 0 can
            # start within ~2us instead of waiting on a serial 6 MB preamble.
            nc.sync.dma_start(out=qT_sb[:, 0], in_=qT[:, 0])
            nc.sync.dma_start(out=kT_sb[:, 0], in_=kT[:, 0])
            nc.scalar.dma_start(out=bv_sb[:], in_=bvs[:])
            for g in range(2):
                nc.scalar.dma_start(out=w1_sb[:, 0, g], in_=w1s[:, 0, g])
            for j in range(2):
                nc.sync.dma_start(out=qhi[0][64:128, j, :], in_=costT[64 * j : 64 * j + 64, :])
            for g in range(2, 8):
                nc.scalar.dma_start(out=w1_sb[:, 0, g], in_=w1s[:, 0, g])
            nc.scalar.dma_start(out=w2_sb[:, 0], in_=w2s[:, 0])
            nc.sync.dma_start(out=qT_sb[:, 1], in_=qT[:, 1])
            nc.sync.dma_start(out=kT_sb[:, 1], in_=kT[:, 1])
            for j in range(2, 4):
                nc.sync.dma_start(out=qhi[0][64:128, j, :], in_=costT[64 * j : 64 * j + 64, :])
            nc.scalar.dma_start(out=vx_sb[:, 0], in_=vx[:, 0])
            for j in range(4, 8):
                nc.sync.dma_start(out=qhi[0][64:128, j, :], in_=costT[64 * j : 64 * j + 64, :])
            nc.scalar.dma_start(out=w1_sb[:, 1], in_=w1s[:, 1])
            nc.scalar.dma_start(out=w2_sb[:, 1], in_=w2s[:, 1])
            for j in range(8):
                nc.sync.dma_start(out=qhi[1][64:128, j, :], in_=costT[64 * j : 64 * j + 64, :])
            nc.sync.dma_start(out=qT_sb[:, 2:], in_=qT[:, 2:])
            nc.sync.dma_start(out=kT_sb[:, 2:], in_=kT[:, 2:])
            nc.scalar.dma_start(out=vx_sb[:, 1], in_=vx[:, 1])
            for hh in range(2, HPC):
                nc.scalar.dma_start(out=w1_sb[:, hh], in_=w1s[:, hh])
                nc.scalar.dma_start(out=w2_sb[:, hh], in_=w2s[:, hh])
                nc.scalar.dma_start(out=vx_sb[:, hh], in_=vx[:, hh])

            relu_rr = itertools.cycle([nc.vector, nc.scalar])
            stage_rr = itertools.cycle([nc.vector, nc.scalar])
            dma_rr = itertools.cycle([nc.sync, nc.gpsimd])

            def emit_st(hs, jb):
                # one 128-c-wide S^T block for head hs into its qhi buffer
                qdst = qhi[hs % 2]
                ps = psSp.tile([128, 512], FP, name="ps", tag="ps")
                nc.tensor.matmul(
                    ps[:],
                    lhsT=kT_sb[:, hs, 128 * jb : 128 * jb + 128],
                    rhs=qT_sb[:, hs, :],
                    start=True,
                    stop=True,
                )
                stg = stgp.tile([128, 512], mm_dt, name="stg", tag="stg")
                eng = next(stage_rr)
                if eng is nc.scalar:
                    nc.scalar.copy(out=stg[:], in_=ps[:])
                else:
                    eng.tensor_copy(out=stg[:], in_=ps[:])
                for half in range(2):
                    next(dma_rr).dma_start(
                        out=qdst[0:64, 2 * jb + half, :],
                        in_=stg[64 * half : 64 * half + 64, :],
                    )

            def emit_relu(r1slot, p1ap, hh):
                eng = next(relu_rr)
                if eng is nc.scalar:
                    nc.scalar.activation(
                        r1slot, p1ap, AF.Relu, bias=bv_sb[:, hh : hh + 1]
                    )
                else:
                    eng.tensor_scalar(
                        out=r1slot,
                        in0=p1ap,
                        scalar1=bv_sb[:, hh : hh + 1],
                        scalar2=0.0,
                        op0=ALU.add,
                        op1=ALU.max,
                    )

            for jb in range(4):
                emit_st(0, jb)

            # PV matmuls for chunk ci are deferred into chunk ci+1 so the PE
            # never waits on the exp; the head's out-copy rides the last flush.
            hold = {"pend": None, "pvT": None}

            def flush_pv():
                if hold["pend"] is None:
                    return
                hh_, ci_, wx_ = hold["pend"]
                hold["pend"] = None
                if ci_ == 0:
                    hold["pvT"] = pspvp.tile([17, 512], FP, name="pvT", tag="pvT")
                pvT = hold["pvT"]
                for jj in range(2):
                    nc.tensor.matmul(
                        pvT[:],
                        lhsT=vx_sb[:, hh_, 2 * ci_ + jj, :],
                        rhs=wx_[:, jj, :],
                        start=(ci_ == 0 and jj == 0),
                        stop=(ci_ == 3 and jj == 1),
                    )
                if ci_ == 3:
                    ot = osbp.tile([17, 512], FP, name="ot", tag="ot")
                    if hh_ % 2 == 0:
                        nc.vector.tensor_copy(out=ot[:], in_=pvT[:])
                    else:
                        nc.scalar.copy(out=ot[:], in_=pvT[:])
                    nc.sync.dma_start(out=outp[hh_], in_=ot[:])

            for hh in range(HPC):
                qh = qhi[hh % 2]
                for ci in range(4):
                    if hh + 1 < HPC:
                        emit_st(hh + 1, ci)
                    pmx = psmxp.tile([64, 2, 512], FP)
                    r1t = [[None, None, None, None] for _ in range(2)]

                    def emit_mix2(pair):
                        # both jj strips back-to-back: shared w2 stationary
                        for jj in range(2):
                            nc.tensor.matmul(
                                pmx[:, jj, :],
                                lhsT=w2_sb[:, hh, pair, :, :],
                                rhs=r1t[jj][pair][:],
                                start=(pair == 0),
                                stop=(pair == 3),
                                perf_mode=DR,
                            )

                    # mix1: g-outer, jj-inner -> each w1 stationary used twice;
                    # single-buffer PSUM tiles (4 bufs) keep a 4-deep pipeline.
                    # relu fills fp8 pair tiles consumed by lagged DoubleRow
                    # mix2 (emitted one g-quad late so relu latency is hidden).
                    for pair in range(4):
                        for jj in range(2):
                            r1t[jj][pair] = r1p.tile(
                                [128, 2, 512], F8, name="r1", tag="r1"
                            )
                        for i in range(2):
                            g = 2 * pair + i
                            p1s = []
                            for jj in range(2):
                                p1 = ps1p.tile([128, 512], FP, name="p1", tag="p1")
                                nc.tensor.matmul(
                                    p1[:],
                                    lhsT=w1_sb[:, hh, g, :],
                                    rhs=qh[:, 2 * ci + jj, :],
                                    start=True,
                                    stop=True,
                                )
                                p1s.append(p1)
                            for jj in range(2):
                                emit_relu(r1t[jj][pair][:, i, :], p1s[jj][:], hh)
                        if pair == 1:
                            flush_pv()
                        if pair >= 1:
                            emit_mix2(pair - 1)
                    emit_mix2(3)
                    wx = wexpp.tile([64, 2, 512], mm_dt, name="wx", tag="wexp")
                    nc.scalar.activation(wx[:], pmx[:], AF.Exp)
                    hold["pend"] = (hh, ci, wx)
            flush_pv()
    _dedupe_weight_loads(nc)
    nc.finalize()
    return nc


def _dedupe_weight_loads(nc):
    """Walk the scheduled PE sequence; when consecutive matmuls use the
    identical stationary AP, mark the later ones ldweights=False so codegen
    skips the redundant LDWEIGHTS (the array still holds those weights)."""
    n = 0
    for bb in nc.m.functions[0].blocks:
        last = None
        for ins in bb.instructions:
            if not isinstance(ins, mybir.InstMatmult):
                continue
            w = ins.ins[1]
            key = (w.memref, w.offset, str(w.ap), str(w.dtype))
            if key == last and ins.ldweights is None:
                ins.ldweights = False
                n += 1
            last = key
    print(f"deduped {n} weight loads", file=sys.stderr)


def prepare_in_maps(q, k, v, cost_mat, mix1_weight, mix1_bias, mix2_weight, mix2_bias):
    q = np.asarray(q, np.float32)
    k = np.asarray(k, np.float32)
    v = np.asarray(v, np.float32)
    cost_mat = np.asarray(cost_mat, np.float32)
    mix1_weight = np.asarray(mix1_weight, np.float32)
    mix1_bias = np.asarray(mix1_bias, np.float32)
    mix2_weight = np.asarray(mix2_weight, np.float32)
    mix2_bias = np.asarray(mix2_bias, np.float32)

    in_maps = []
    for core in range(NCORES):
        b = core // 2
        h0 = (core % 2) * HPC
        qT = np.ascontiguousarray(q[b, h0 : h0 + HPC].transpose(2, 0, 1)) * 0.25
        kT = np.ascontiguousarray(k[b, h0 : h0 + HPC].transpose(2, 0, 1))
        costT = np.ascontiguousarray(cost_mat[b].T)
        vv = v[b, h0 : h0 + HPC]  # (HPC, C, D)
        vxa = np.empty((64, HPC, 8, 17), np.float32)
        vxa[:, :, :, :16] = vv.reshape(HPC, 8, 64, 16).transpose(2, 0, 1, 3)
        vxa[:, :, :, 16] = 1.0

        w1 = mix1_weight[h0 : h0 + HPC]  # (HPC, 2, M)
        b1 = mix1_bias[h0 : h0 + HPC]  # (HPC, M)
        w2 = mix2_weight[h0 : h0 + HPC, :, 0]  # (HPC, M)
        aw = np.abs(w2)
        sg = np.sign(w2).astype(np.float32)
        A = (w1[:, 0, :] * aw).astype(np.float32)  # (HPC, M)
        Cc = (w1[:, 1, :] * aw).astype(np.float32)
        Bb = (b1 * aw).astype(np.float32)

        w1s = np.zeros((128, HPC, 8, 128), np.float32)
        for g in range(8):
            for c8 in range(8):
                cols = slice(c8 * 16, c8 * 16 + 16)
                w1s[8 * g + c8, :, g, cols] = A
                w1s[64 + 8 * g + c8, :, g, cols] = Cc
        w2s = np.zeros((128, HPC, 4, 2, 64), np.float32)
        for pair in range(4):
            for i in range(2):
                g = 2 * pair + i
                for c8 in range(8):
                    w2s[c8 * 16 : c8 * 16 + 16, :, pair, i, 8 * g + c8] = sg.T
        w2s = w2s.astype(ml_dtypes.float8_e4m3)
        bvs = np.tile(Bb.T, (8, 1)).astype(np.float32)  # (128, HPC)

        in_maps.append(
            dict(qT=qT, kT=kT, costT=costT, vx=vxa, w1s=w1s, w2s=w2s, bvs=bvs)
        )
    return in_maps


def assemble(results):
    full = np.empty((B, R, H * D), np.float32)
    for core in range(NCORES):
        b = core // 2
        c0 = (core % 2) * HPC * D
        o = results[core]["out"]  # (HPC, D+1, R); row D is the softmax denom
        o = o[:, :D, :] / o[:, D : D + 1, :]
        full[b, :, c0 : c0 + HPC * D] = o.transpose(2, 0, 1).reshape(R, HPC * D)
    return full


_nc_cache = None


def _install_ntff_hook():
    """The agent image's antenv lacks axon_hooks; recreate it and register
    the ctypes NTFF profiling hook so trace=True yields exec times."""
    import types

    try:
        import antenv

        try:
            import antenv.axon_hooks  # noqa: F401

            return
        except ImportError:
            pass
        mod = types.ModuleType("antenv.axon_hooks")
        mod._hook = None
        mod.set_axon_ntff_profile_hook = lambda h: setattr(mod, "_hook", h)
        mod.get_axon_ntff_profile_hook = lambda: mod._hook
        sys.modules["antenv.axon_hooks"] = mod
        antenv.axon_hooks = mod
        from trn_agent_boot.trn_boot import _ntff_profile_via_ctypes

        mod._hook = _ntff_profile_via_ctypes("/opt/axon/libaxon_pjrt.so")
    except Exception as e:  # profiling is best-effort
        print(f"ntff hook install failed: {e}", file=sys.stderr)


def kernel(**inputs) -> np.ndarray:
    global _nc_cache, last_results
    if _nc_cache is None:
        _nc_cache = build_bass()
    in_maps = prepare_in_maps(**inputs)
    trace = bool(int(os.environ.get("KERNEL_TRACE", "0")))
    if trace:
        _install_ntff_hook()
        import concourse.bass_utils as bu

        bu.upload_artifacts = lambda tmpdir: f"local:{tmpdir}"
    res = run_bass_kernel_spmd(_nc_cache, in_maps, list(range(NCORES)), trace=trace)
    last_results = res
    return assemble(res.results)


# revision 13
# speedup vs baseline: 2.5154x; 1.0263x over previous
"""Trainium2 Bass kernel: MixedScore MultiHeadAttention.

Math (per batch b, head h):
  S[r,c]   = (q[b,h,r,:] . k[b,h,c,:]) / 4
  t_m[r,c] = a_m*S + c_m*Q + b1_m          (Q = cost_mat[b])
  mixed    = sum_m w2_m * relu(t_m)  (+ b2, dropped: softmax shift-invariant)
  out      = softmax_c(mixed) @ v

Folding |w2_m| into (a_m, c_m, b1_m) gives  mixed = sum_m s_m * relu(A_m*S + C_m*Q + B_m)
with s_m = sign(w2_m), so the w2 multiply disappears.

Layout strategy (per core; core = (b, half-of-heads) shard, 8 heads/core):
  - Everything transposed: S^T tiles (c on partitions, r in free dim).
  - qhi SBUF tensor (128, 8, 512): partitions 0:64 = S^T 64-row c-chunk
    (rewritten per head), partitions 64:128 = cost^T rows (DMA'd once).
    S^T emitted 128-c-wide (4 matmuls/head) then split into two 64-row
    copies on rotating engines.
  - mix1: per (ci, jj): 8 K=128 matmuls (one per g-group of 8 c-values),
    g-PAIRS written into one PSUM tile (128p=(c8,m), 2, 512r).
  - relu with per-partition bias B_m in ONE op per pair (128x1024) on a
    rotating engine (Pool/ACT/DVE), output fp8e4m3 -> paired SBUF tile.
  - mix2: fp8 DoubleRow matmul per pair: lhsT = (128,2,64) sign pattern,
    rhs = (128,2,512) relu pair; 2 k-tiles per pass at 0.5 cyc/row ->
    4x fewer PE cycles than the fp32 version. Accumulates (64c, 512r)
    mixed^T strips in PSUM over the 4 pairs.
  - exp on ACT (no max subtraction: |logit| <= ~21, fp32-safe).
  - PV: lhsT = [v | ones] (c, 17), rhs = exp'd weights (c-chunk, 512r);
    col 16 accumulates the softmax denominator; divide on host.
Matmuls use float32r (full-rate); mix2 uses fp8 DoubleRow (2x rate).
fp8 quantization of relu outputs costs ~8e-3 end-to-end rel err (gate 2e-2).
"""

import itertools
import os
import sys

import ml_dtypes
import numpy as np

sys.path.insert(0, "/opt/trn_rl_repo")

import concourse.bass as bass  # noqa: E402
import concourse.mybir as mybir  # noqa: E402
from concourse import bacc, tile  # noqa: E402
from concourse.bass_utils import run_bass_kernel_spmd  # noqa: E402

FP = mybir.dt.float32
FPR = mybir.dt.float32r
F8 = mybir.dt.float8e4
BF16 = mybir.dt.bfloat16
B, H, R, C, D, M = 4, 16, 512, 512, 16, 16
HPC = 8  # heads per core
NCORES = 8

AF = mybir.ActivationFunctionType
ALU = mybir.AluOpType
DR = mybir.MatmulPerfMode.DoubleRow

last_results = None  # BassKernelResults of the most recent run (for test.py)


def build_bass(mm_dt=FPR, w1_dt=FPR):
    nc = bacc.Bacc(None, target_bir_lowering=False, debug=False)

    qT = nc.declare_dram_parameter("qT", [D, HPC, R], mm_dt, isOutput=False)
    kT = nc.declare_dram_parameter("kT", [D, HPC, C], mm_dt, isOutput=False)
    costT = nc.declare_dram_parameter("costT", [C, R], mm_dt, isOutput=False)
    vx = nc.declare_dram_parameter("vx", [64, HPC, 8, 17], mm_dt, isOutput=False)
    w1s = nc.declare_dram_parameter("w1s", [128, HPC, 8, 128], w1_dt, isOutput=False)
    w2s = nc.declare_dram_parameter("w2s", [128, HPC, 4, 2, 64], F8, isOutput=False)
    bvs = nc.declare_dram_parameter("bvs", [128, HPC], FP, isOutput=False)
    outp = nc.declare_dram_parameter("out", [HPC, D + 1, R], FP, isOutput=True)

    with tile.TileContext(nc) as tc:
        with (
            tc.tile_pool(name="const", bufs=1) as constp,
            tc.tile_pool(name="qhi", bufs=1) as qhip,
            tc.tile_pool(name="r1", bufs=8) as r1p,
            tc.tile_pool(name="wexp", bufs=4) as wexpp,
            tc.tile_pool(name="osb", bufs=4) as osbp,
            tc.tile_pool(name="stg", bufs=3) as stgp,
            tc.tile_pool(name="psS", bufs=1, space="PSUM") as psSp,
            tc.tile_pool(name="ps1", bufs=4, space="PSUM") as ps1p,
            tc.tile_pool(name="psmx", bufs=1, space="PSUM") as psmxp,
            tc.tile_pool(name="pspv", bufs=1, space="PSUM") as pspvp,
        ):
            w1_sb = constp.tile([128, HPC, 8, 128], w1_dt)
            w2_sb = constp.tile([128, HPC, 4, 2, 64], F8)
            bv_sb = constp.tile([128, HPC], FP)
            qT_sb = constp.tile([D, HPC, R], mm_dt)
            kT_sb = constp.tile([D, HPC, C], mm_dt)
            vx_sb = constp.tile([64, HPC, 8, 17], mm_dt)

            qhi = [qhip.tile([128, 8, 512], mm_dt, name=f"qhi{i}", tag=f"qhi{i}") for i in range(2)]
            # Preamble DMAs ordered by first use, split across the two HWDGE
            # queues (SP carries cost/qhi, ACT carries weights) so head 0 can
            # start within ~2us instead of waiting on a serial 6 MB preamble.
            nc.sync.dma_start(out=qT_sb[:, 0], in_=qT[:, 0])
            nc.sync.dma_start(out=kT_sb[:, 0], in_=kT[:, 0])
            nc.scalar.dma_start(out=w1_sb[:, 0], in_=w1s[:, 0])
            nc.scalar.dma_start(out=bv_sb[:], in_=bvs[:])
            nc.scalar.dma_start(out=w2_sb[:, 0], in_=w2s[:, 0])
            for j in range(8):
                nc.sync.dma_start(out=qhi[0][64:128, j, :], in_=costT[64 * j : 64 * j + 64, :])
            nc.scalar.dma_start(out=vx_sb[:, 0], in_=vx[:, 0])
            nc.scalar.dma_start(out=qT_sb[:, 1:], in_=qT[:, 1:])
            nc.scalar.dma_start(out=kT_sb[:, 1:], in_=kT[:, 1:])
            for j in range(8):
                nc.sync.dma_start(out=qhi[1][64:128, j, :], in_=costT[64 * j : 64 * j + 64, :])
            for hh in range(1, HPC):
                nc.scalar.dma_start(out=w1_sb[:, hh], in_=w1s[:, hh])
                nc.scalar.dma_start(out=w2_sb[:, hh], in_=w2s[:, hh])
                nc.scalar.dma_start(out=vx_sb[:, hh], in_=vx[:, hh])

            relu_rr = itertools.cycle([nc.vector, nc.scalar])
            stage_rr = itertools.cycle([nc.vector, nc.scalar])
            dma_rr = itertools.cycle([nc.sync, nc.gpsimd])

            def emit_st(hs, jb):
                # one 128-c-wide S^T block for head hs into its qhi buffer
                qdst = qhi[hs % 2]
                ps = psSp.tile([128, 512], FP, name="ps", tag="ps")
                nc.tensor.matmul(
                    ps[:],
                    lhsT=kT_sb[:, hs, 128 * jb : 128 * jb + 128],
                    rhs=qT_sb[:, hs, :],
                    start=True,
                    stop=True,
                )
                stg = stgp.tile([128, 512], mm_dt, name="stg", tag="stg")
                eng = next(stage_rr)
                if eng is nc.scalar:
                    nc.scalar.copy(out=stg[:], in_=ps[:])
                else:
                    eng.tensor_copy(out=stg[:], in_=ps[:])
                for half in range(2):
                    next(dma_rr).dma_start(
                        out=qdst[0:64, 2 * jb + half, :],
                        in_=stg[64 * half : 64 * half + 64, :],
                    )

            def emit_relu(r1slot, p1ap, hh):
                eng = next(relu_rr)
                if eng is nc.scalar:
                    nc.scalar.activation(
                        r1slot, p1ap, AF.Relu, bias=bv_sb[:, hh : hh + 1]
                    )
                else:
                    eng.tensor_scalar(
                        out=r1slot,
                        in0=p1ap,
                        scalar1=bv_sb[:, hh : hh + 1],
                        scalar2=0.0,
                        op0=ALU.add,
                        op1=ALU.max,
                    )

            for jb in range(4):
                emit_st(0, jb)

            # PV matmuls for chunk ci are deferred into chunk ci+1 so the PE
            # never waits on the exp; the head's out-copy rides the last flush.
            hold = {"pend": None, "pvT": None}

            def flush_pv():
                if hold["pend"] is None:
                    return
                hh_, ci_, wx_ = hold["pend"]
                hold["pend"] = None
                if ci_ == 0:
                    hold["pvT"] = pspvp.tile([17, 512], FP, name="pvT", tag="pvT")
                pvT = hold["pvT"]
                for jj in range(2):
                    nc.tensor.matmul(
                        pvT[:],
                        lhsT=vx_sb[:, hh_, 2 * ci_ + jj, :],
                        rhs=wx_[:, jj, :],
                        start=(ci_ == 0 and jj == 0),
                        stop=(ci_ == 3 and jj == 1),
                    )
                if ci_ == 3:
                    ot = osbp.tile([17, 512], FP, name="ot", tag="ot")
                    if hh_ % 2 == 0:
                        nc.vector.tensor_copy(out=ot[:], in_=pvT[:])
                    else:
                        nc.scalar.copy(out=ot[:], in_=pvT[:])
                    nc.sync.dma_start(out=outp[hh_], in_=ot[:])

            for hh in range(HPC):
                qh = qhi[hh % 2]
                for ci in range(4):
                    if hh + 1 < HPC:
                        emit_st(hh + 1, ci)
                    pmx = psmxp.tile([64, 2, 512], FP)
                    r1t = [[None, None, None, None] for _ in range(2)]

                    def emit_mix2(pair):
                        # both jj strips back-to-back: shared w2 stationary
                        for jj in range(2):
                            nc.tensor.matmul(
                                pmx[:, jj, :],
                                lhsT=w2_sb[:, hh, pair, :, :],
                                rhs=r1t[jj][pair][:],
                                start=(pair == 0),
                                stop=(pair == 3),
                                perf_mode=DR,
                            )

                    # mix1: g-outer, jj-inner -> each w1 stationary used twice;
                    # single-buffer PSUM tiles (4 bufs) keep a 4-deep pipeline.
                    # relu fills fp8 pair tiles consumed by lagged DoubleRow
                    # mix2 (emitted one g-quad late so relu latency is hidden).
                    for pair in range(4):
                        for jj in range(2):
                            r1t[jj][pair] = r1p.tile(
                                [128, 2, 512], F8, name="r1", tag="r1"
                            )
                        for i in range(2):
                            g = 2 * pair + i
                            p1s = []
                            for jj in range(2):
                                p1 = ps1p.tile([128, 512], FP, name="p1", tag="p1")
                                nc.tensor.matmul(
                                    p1[:],
                                    lhsT=w1_sb[:, hh, g, :],
                                    rhs=qh[:, 2 * ci + jj, :],
                                    start=True,
                                    stop=True,
                                )
                                p1s.append(p1)
                            for jj in range(2):
                                emit_relu(r1t[jj][pair][:, i, :], p1s[jj][:], hh)
                        if pair == 1:
                            flush_pv()
                        if pair >= 1:
                            emit_mix2(pair - 1)
                    emit_mix2(3)
                    wx = wexpp.tile([64, 2, 512], mm_dt, name="wx", tag="wexp")
                    nc.scalar.activation(wx[:], pmx[:], AF.Exp)
                    hold["pend"] = (hh, ci, wx)
            flush_pv()
    _dedupe_weight_loads(nc)
    nc.finalize()
    return nc


def _dedupe_weight_loads(nc):
    """Walk the scheduled PE sequence; when consecutive matmuls use the
    identical stationary AP, mark the later ones ldweights=False so codegen
    skips the redundant LDWEIGHTS (the array still holds those weights)."""
    n = 0
    for bb in nc.m.functions[0].blocks:
        last = None
        for ins in bb.instructions:
            if not isinstance(ins, mybir.InstMatmult):
                continue
            w = ins.ins[1]
            key = (w.memref, w.offset, str(w.ap), str(w.dtype))
            if key == last and ins.ldweights is None:
                ins.ldweights = False
                n += 1
            last = key
    print(f"deduped {n} weight loads", file=sys.stderr)


def prepare_in_maps(q, k, v, cost_mat, mix1_weight, mix1_bias, mix2_weight, mix2_bias):
    q = np.asarray(q, np.float32)
    k = np.asarray(k, np.float32)
    v = np.asarray(v, np.float32)
    cost_mat = np.asarray(cost_mat, np.float32)
    mix1_weight = np.asarray(mix1_weight, np.float32)
    mix1_bias = np.asarray(mix1_bias, np.float32)
    mix2_weight = np.asarray(mix2_weight, np.float32)
    mix2_bias = np.asarray(mix2_bias, np.float32)

    in_maps = []
    for core in range(NCORES):
        b = core // 2
        h0 = (core % 2) * HPC
        qT = np.ascontiguousarray(q[b, h0 : h0 + HPC].transpose(2, 0, 1)) * 0.25
        kT = np.ascontiguousarray(k[b, h0 : h0 + HPC].transpose(2, 0, 1))
        costT = np.ascontiguousarray(cost_mat[b].T)
        vv = v[b, h0 : h0 + HPC]  # (HPC, C, D)
        vxa = np.empty((64, HPC, 8, 17), np.float32)
        vxa[:, :, :, :16] = vv.reshape(HPC, 8, 64, 16).transpose(2, 0, 1, 3)
        vxa[:, :, :, 16] = 1.0

        w1 = mix1_weight[h0 : h0 + HPC]  # (HPC, 2, M)
        b1 = mix1_bias[h0 : h0 + HPC]  # (HPC, M)
        w2 = mix2_weight[h0 : h0 + HPC, :, 0]  # (HPC, M)
        aw = np.abs(w2)
        sg = np.sign(w2).astype(np.float32)
        A = (w1[:, 0, :] * aw).astype(np.float32)  # (HPC, M)
        Cc = (w1[:, 1, :] * aw).astype(np.float32)
        Bb = (b1 * aw).astype(np.float32)

        w1s = np.zeros((128, HPC, 8, 128), np.float32)
        for g in range(8):
            for c8 in range(8):
                cols = slice(c8 * 16, c8 * 16 + 16)
                w1s[8 * g + c8, :, g, cols] = A
                w1s[64 + 8 * g + c8, :, g, cols] = Cc
        w2s = np.zeros((128, HPC, 4, 2, 64), np.float32)
        for pair in range(4):
            for i in range(2):
                g = 2 * pair + i
                for c8 in range(8):
                    w2s[c8 * 16 : c8 * 16 + 16, :, pair, i, 8 * g + c8] = sg.T
        w2s = w2s.astype(ml_dtypes.float8_e4m3)
        bvs = np.tile(Bb.T, (8, 1)).astype(np.float32)  # (128, HPC)

        in_maps.append(
            dict(qT=qT, kT=kT, costT=costT, vx=vxa, w1s=w1s, w2s=w2s, bvs=bvs)
        )
    return in_maps


def assemble(results):
    full = np.empty((B, R, H * D), np.float32)
    for core in range(NCORES):
        b = core // 2
        c0 = (core % 2) * HPC * D
        o = results[core]["out"]  # (HPC, D+1, R); row D is the softmax denom
        o = o[:, :D, :] / o[:, D : D + 1, :]
        full[b, :, c0 : c0 + HPC * D] = o.transpose(2, 0, 1).reshape(R, HPC * D)
    return full


_nc_cache = None


def _install_ntff_hook():
    """The agent image's antenv lacks axon_hooks; recreate it and register
    the ctypes NTFF profiling hook so trace=True yields exec times."""
    import types

    try:
        import antenv

        try:
            import antenv.axon_hooks  # noqa: F401

            return
        except ImportError:
            pass
        mod = types.ModuleType("antenv.axon_hooks")
        mod._hook = None
        mod.set_axon_ntff_profile_hook = lambda h: setattr(mod, "_hook", h)
        mod.get_axon_ntff_profile_hook = lambda: mod._hook
        sys.modules["antenv.axon_hooks"] = mod
        antenv.axon_hooks = mod
        from trn_agent_boot.trn_boot import _ntff_profile_via_ctypes

        mod._hook = _ntff_profile_via_ctypes("/opt/axon/libaxon_pjrt.so")
    except Exception as e:  # profiling is best-effort
        print(f"ntff hook install failed: {e}", file=sys.stderr)


def kernel(**inputs) -> np.ndarray:
    global _nc_cache, last_results
    if _nc_cache is None:
        _nc_cache = build_bass()
    in_maps = prepare_in_maps(**inputs)
    trace = bool(int(os.environ.get("KERNEL_TRACE", "0")))
    if trace:
        _install_ntff_hook()
        import concourse.bass_utils as bu

        bu.upload_artifacts = lambda tmpdir: f"local:{tmpdir}"
    res = run_bass_kernel_spmd(_nc_cache, in_maps, list(range(NCORES)), trace=trace)
    last_results = res
    return assemble(res.results)
